# revision 22
# baseline (speedup 1.0000x reference)
"""Trainium2 8-core Bass kernel for AdaptiveAttentionTransformerBlock.

Sparse attention with a latent (stride-64 mean-pooled KV) branch and a local
sliding-window (width 64) branch, concatenated per head and mixed by w_mix.

Sharding: 16 heads -> 2 heads per core (all 8 cores), both batches per core.
Each core computes qkv + RoPE + both attention branches for its 2 heads in a
transposed [feature, seq] layout; two 8-rank AllToAll chunks (one per local
head) redistribute from head-sharding to (batch, seq/4) sharding, overlapped
with the second head's attention; each core then computes the full mix
projection for its 512-token slice in two accumulation waves. The host
concatenates the 8 slices.

Self-contained: hardcodes shapes B=2, S=2048, E=1024, H=16, D=64, stride=64.
"""

import os
import sys

import numpy as np

for _p in ("/opt/trn_rl_repo",):
    if _p not in sys.path and os.path.isdir(_p):
        sys.path.insert(0, _p)

import ml_dtypes  # noqa: E402

B, S, E, H, D = 2, 2048, 1024, 16, 64
STRIDE, C = 64, 32
NCORES, HLOC = 8, 2
SB = S // 4  # 512, per-core output seq slice
NT = S // 128  # 16 q/k tiles
BF16 = ml_dtypes.bfloat16

_BUILD_CACHE = {}


# ----------------------------------------------------------------------------
# host-side constants
# ----------------------------------------------------------------------------

def _host_constants():
    inv = 1.0 / (10000.0 ** (np.arange(0, D, 2, dtype=np.float64) / D))
    pos = np.arange(S, dtype=np.float64)
    ang = pos[None, :] * inv[(np.arange(D) % (D // 2))][:, None]  # [64, S]
    cosT = np.cos(ang).astype(np.float32)
    sinT = np.sin(ang).astype(np.float32)
    cos2 = np.vstack([cosT, cosT])  # [128, S] f32
    sin2 = np.vstack([sinT, sinT])

    L = np.zeros((D, D), np.float32)
    for j in range(D // 2):
        L[2 * j + 1, 2 * j] = -1.0
        L[2 * j, 2 * j + 1] = 1.0
    L2 = np.zeros((2 * D, 2 * D), np.float32)
    L2[:D, :D] = L
    L2[D:, D:] = L

    # combined local mask for scores_T of key-tile kt vs q-tiles [kt, kt+1]:
    # cols 0:128 (q in same tile): key j visible iff j <= i <= j+63
    # cols 128:256 (q in next tile): key j visible iff j >= i + 65
    i = np.arange(128)
    mdiag = ((i[None, :] >= i[:, None]) & (i[None, :] - i[:, None] <= 63)).astype(np.float32)
    mprev = (i[:, None] >= i[None, :] + 65).astype(np.float32)
    mloc = np.concatenate([mdiag, mprev], axis=1)  # [128, 256]

    cc = np.arange(C)
    ss = np.arange(S)
    latm = (cc[:, None] * STRIDE <= ss[None, :]).astype(np.float32)  # [32, S]

    A = np.zeros((S, C), np.float32)
    for c in range(C):
        A[c * STRIDE:(c + 1) * STRIDE, c] = 1.0 / STRIDE

    ident = np.eye(128, dtype=np.float32)
    return {
        "cos2": cos2,
        "sin2": sin2,
        "r2": L2.astype(BF16),
        "mloc": mloc.astype(BF16),
        "latm": latm.astype(BF16),
        "amat": np.ascontiguousarray(
            A.reshape(16, 128, C).transpose(1, 0, 2)).astype(BF16),
        "identf": ident,
        "identb": ident.astype(BF16),
    }


# ----------------------------------------------------------------------------
# bass graph
# ----------------------------------------------------------------------------

def build_bass():
    import concourse.mybir as mb
    import concourse.tile as tile
    from concourse import bacc

    f32 = mb.dt.float32
    bf16 = mb.dt.bfloat16

    nc = bacc.Bacc(
        "TRN2",
        target_bir_lowering=False,
        debug=False,
        num_devices=NCORES,
    )

    xT_d = nc.declare_dram_parameter("xT", [8, 128, 8, 512], bf16, isOutput=False)
    wqkT_d = nc.declare_dram_parameter("wqkT", [128, 8, 256], bf16, isOutput=False)
    wvT_d = nc.declare_dram_parameter("wvT", [128, 8, 128], bf16, isOutput=False)
    # w_mix.T with rows permuted into the two A2A wave orders (even heads, odd heads)
    wmixT_d = nc.declare_dram_parameter("wmixT", [128, 16, E], bf16, isOutput=False)
    cos2_d = nc.declare_dram_parameter("cos2", [128, S], f32, isOutput=False)
    identb_d = nc.declare_dram_parameter("identb", [128, 128], bf16, isOutput=False)
    sin2_d = nc.declare_dram_parameter("sin2", [128, S], f32, isOutput=False)
    r2_d = nc.declare_dram_parameter("r2", [128, 128], bf16, isOutput=False)
    mloc_d = nc.declare_dram_parameter("mloc", [128, 256], bf16, isOutput=False)
    latm_d = nc.declare_dram_parameter("latm", [C, S], bf16, isOutput=False)
    amat_d = nc.declare_dram_parameter("amat", [128, 16, C], bf16, isOutput=False)
    y_d = nc.declare_dram_parameter("y", [SB, E], f32, isOutput=True)

    SCALE = 1.0 / 8.0  # 1/sqrt(D)

    with tile.TileContext(nc, num_cores=NCORES) as tc:
        with (
            tc.tile_pool(name="const", bufs=1) as constp,
            tc.tile_pool(name="xt", bufs=1) as xtp,
            tc.tile_pool(name="qk", bufs=1) as qkp,
            tc.tile_pool(name="vex", bufs=1) as vexp,
            tc.tile_pool(name="lat", bufs=1) as latp,
            tc.tile_pool(name="plat", bufs=3) as platp,
            tc.tile_pool(name="pk", bufs=3) as pkp,
            tc.tile_pool(name="tmp", bufs=3) as tmpp,
            tc.tile_pool(name="oc", bufs=4) as ocp,
            tc.tile_pool(name="ysb", bufs=1) as ysbp,
            tc.tile_pool(name="psq", bufs=2, space="PSUM") as psq,
            tc.tile_pool(name="psm", bufs=6, space="PSUM") as psm,
            tc.tile_pool(name="dram", bufs=1, space="DRAM") as dramp,
        ):
            # ---- critical-path loads first: qk weights + x chunks ----
            wqk_sb = constp.tile([128, 8, 256], bf16, name="wqk_sb")
            nc.sync.dma_start(wqk_sb[:], wqkT_d.ap())
            xt_tiles = []
            for i in range(8):
                t = xtp.tile([128, 8, 512], bf16, name=f"xt{i}", tag="xt", bufs=6)
                nc.sync.dma_start(t[:], xT_d.ap()[i])
                xt_tiles.append(t)
            wv_sb = constp.tile([128, 8, 128], bf16, name="wv_sb")
            nc.sync.dma_start(wv_sb[:], wvT_d.ap())
            r2_sb = constp.tile([128, 128], bf16, name="r2_sb")
            nc.gpsimd.dma_start(r2_sb[:], r2_d.ap())
            cos_sb = constp.tile([128, S], f32, name="cos_sb")
            nc.gpsimd.dma_start(cos_sb[:], cos2_d.ap())
            sin_sb = constp.tile([128, S], f32, name="sin_sb")
            nc.gpsimd.dma_start(sin_sb[:], sin2_d.ap())
            identb_sb = constp.tile([128, 128], bf16, name="identb_sb")
            nc.gpsimd.dma_start(identb_sb[:], identb_d.ap())
            amat_sb = constp.tile([128, 16, C], bf16, name="amat_sb")
            nc.gpsimd.dma_start(amat_sb[:], amat_d.ap())
            mloc_sb = constp.tile([128, 256], bf16, name="mloc_sb")
            nc.gpsimd.dma_start(mloc_sb[:], mloc_d.ap())
            latm_sb = constp.tile([C, S], bf16, name="latm_sb")
            nc.gpsimd.dma_start(latm_sb[:], latm_d.ap())
            wmix_sb = constp.tile([128, 16, E], bf16, name="wmix_sb")
            nc.sync.dma_start(wmix_sb[:], wmixT_d.ap())

            a2a_in = [dramp.tile([NCORES, SB, 128], bf16, name=f"a2a_in{h}") for h in range(HLOC)]
            a2a_out = [dramp.tile([NCORES, SB, 128], bf16, name=f"a2a_out{h}") for h in range(HLOC)]

            q_rot, k_rot, v_ext, klat2 = [], [], [], []
            vlat = [[None] * HLOC for _ in range(B)]

            # ================= qkv + rope (both batches) =================
            for b in range(B):
                q_rot.append(qkp.tile([128, S], bf16, name=f"q_rot{b}", tag=f"q_rot{b}"))
                k_rot.append(qkp.tile([128, S], bf16, name=f"k_rot{b}", tag=f"k_rot{b}"))
                pending_rope = []

                def _rope_tail(dst, sc, ps_qk):
                    tmp_bf = tmpp.tile([128, 512], bf16, name="tmp_bf", tag="tmp_bf")
                    nc.any.tensor_copy(tmp_bf[:], ps_qk[:])
                    ps_rh = psm.tile([128, 512], f32, name="ps_rh", tag="psm")
                    nc.tensor.matmul(ps_rh[:], r2_sb[:], tmp_bf[:], start=True, stop=True)
                    ssl = slice(sc * 512, (sc + 1) * 512)
                    t1 = tmpp.tile([128, 512], f32, name="t1", tag="t1")
                    nc.vector.tensor_mul(t1[:], ps_qk[:], cos_sb[:, ssl])
                    t2 = tmpp.tile([128, 512], f32, name="t2", tag="t2")
                    nc.vector.tensor_mul(t2[:], ps_rh[:], sin_sb[:, ssl])
                    nc.vector.tensor_add(dst[:, ssl], t1[:], t2[:])

                for jt in range(2):  # 0: q, 1: k
                    dst = (q_rot if jt == 0 else k_rot)[b]
                    for sc in range(4):
                        xt = xt_tiles[b * 4 + sc]
                        ps_qk = psq.tile([128, 512], f32, name="ps_qk", tag="psq")
                        for e in range(8):
                            nc.tensor.matmul(
                                ps_qk[:],
                                wqk_sb[:, e, jt * 128:(jt + 1) * 128],
                                xt[:, e, :],
                                start=(e == 0),
                                stop=(e == 7),
                            )
                        pending_rope.append((dst, sc, ps_qk))
                        if len(pending_rope) > 1:
                            _rope_tail(*pending_rope.pop(0))
                while pending_rope:
                    _rope_tail(*pending_rope.pop(0))

                # v^T then PE-transpose into v_ext [s-part, 16, 2 heads, 64+1]
                ve = vexp.tile([128, 16, HLOC, 65], bf16, name=f"v_ext{b}", tag=f"v_ext{b}")
                v_ext.append(ve)
                nc.any.memset(ve[:, :, :, 64], 1.0)
                pending_vt = []

                def _v_tail(sc, vT_bf):
                    for sub in range(4):
                        st = sc * 4 + sub
                        ps_tv = psm.tile([128, 128], bf16, name="ps_tv", tag="psm")
                        nc.tensor.transpose(
                            ps_tv[:], vT_bf[:, sub * 128:(sub + 1) * 128], identb_sb[:]
                        )
                        nc.any.tensor_copy(
                            ve[:, st, :, 0:64],
                            ps_tv[:].rearrange("p (h d) -> p h d", h=2),
                        )

                for sc in range(4):
                    xt = xt_tiles[b * 4 + sc]
                    ps_vT = psq.tile([128, 512], f32, name="ps_vT", tag="psq")
                    for e in range(8):
                        nc.tensor.matmul(
                            ps_vT[:],
                            wv_sb[:, e, :],
                            xt[:, e, :],
                            start=(e == 0),
                            stop=(e == 7),
                        )
                    vT_bf = tmpp.tile([128, 512], bf16, name="vT_bf", tag="vT_bf")
                    nc.any.tensor_copy(vT_bf[:], ps_vT[:])
                    pending_vt.append((sc, vT_bf))
                    if len(pending_vt) > 1:
                        _v_tail(*pending_vt.pop(0))
                while pending_vt:
                    _v_tail(*pending_vt.pop(0))

                # latent k: chunk means for both heads at their partition offsets
                kl2 = latp.tile([128, C], bf16, name=f"klat2{b}", tag=f"klat2{b}")
                klat2.append(kl2)
                kl32 = latp.tile([128, C], f32, name=f"kl32{b}", tag="kl32")
                nc.vector.tensor_reduce(
                    out=kl32[:],
                    in_=k_rot[b][:].rearrange("p (c w) -> p c w", w=STRIDE),
                    op=mb.AluOpType.add,
                    axis=mb.AxisListType.X,
                )
                nc.scalar.activation(kl2[:], kl32[:], mb.ActivationFunctionType.Copy,
                                     bias=0.0, scale=1.0 / STRIDE)

                # latent v (+ones col) per head: vlat_ext[c, 65] via A-matmul
                for hh in range(HLOC):
                    ps_vl = psm.tile([C, 65], f32, name="ps_vl", tag="psm")
                    for st in range(16):
                        nc.tensor.matmul(
                            ps_vl[:],
                            amat_sb[:, st, :],
                            v_ext[b][:, st, hh, :],
                            start=(st == 0),
                            stop=(st == 15),
                        )
                    vl = latp.tile([C, 65], bf16, name=f"vlat{b}{hh}", tag=f"vlat{b}{hh}")
                    nc.any.tensor_copy(vl[:], ps_vl[:])
                    vlat[b][hh] = vl

            # ================= attention (hh-major) + chunked A2A + mix ====
            y0_sb = []  # wave-0 partial mix results
            for hh in range(HLOC):
                hsl = slice(hh * D, (hh + 1) * D)
                for b in range(B):
                    # latent probabilities for all queries of this (b, head)
                    p_lat = platp.tile([C, S], bf16, name="p_lat", tag="p_lat")
                    for qc in range(4):
                        qsl512 = slice(qc * 512, (qc + 1) * 512)
                        ps_ls = psm.tile([C, 512], f32, name="ps_ls", tag="psm")
                        nc.tensor.matmul(ps_ls[:], klat2[b][hsl, :], q_rot[b][hsl, qsl512],
                                         start=True, stop=True)
                        nc.scalar.activation(p_lat[:, qsl512], ps_ls[:],
                                             mb.ActivationFunctionType.Exp,
                                             bias=0.0, scale=SCALE)
                        nc.vector.tensor_mul(p_lat[:, qsl512], p_lat[:, qsl512],
                                             latm_sb[:, qsl512])

                    # software-pipelined: scores one tile ahead, output one behind
                    pk = {}

                    def _scores(kt):
                        qn = 256 if kt < NT - 1 else 128
                        ps_s = psm.tile([128, 256], f32, name="ps_s", tag="psm")
                        nc.tensor.matmul(
                            ps_s[:, :qn],
                            k_rot[b][hsl, kt * 128:(kt + 1) * 128],
                            q_rot[b][hsl, kt * 128:kt * 128 + qn],
                            start=True, stop=True,
                        )
                        p_kt = pkp.tile([128, 256], bf16, name="p_kt", tag="p_kt")
                        nc.scalar.activation(p_kt[:, :qn], ps_s[:, :qn],
                                             mb.ActivationFunctionType.Exp,
                                             bias=0.0, scale=SCALE)
                        nc.vector.tensor_mul(p_kt[:, :qn], p_kt[:, :qn], mloc_sb[:, :qn])
                        pk[kt] = p_kt

                    def _av(qt):
                        qsl = slice(qt * 128, (qt + 1) * 128)
                        ps_o = psm.tile([128, 130], f32, name="ps_o", tag="psm")
                        nc.tensor.matmul(ps_o[:, 0:65], p_lat[:, qsl], vlat[b][hh][:],
                                         start=True, stop=True)
                        if qt > 0:
                            nc.tensor.matmul(ps_o[:, 65:130], pk[qt - 1][:, 128:256],
                                             v_ext[b][:, qt - 1, hh, :],
                                             start=True, stop=False)
                        nc.tensor.matmul(ps_o[:, 65:130], pk[qt][:, 0:128],
                                         v_ext[b][:, qt, hh, :],
                                         start=(qt == 0), stop=True)
                        if qt > 0:
                            del pk[qt - 1]
                        # normalize -> ocat [128 q, 128 (cmp|loc)] bf16 (DVE only)
                        rec2 = tmpp.tile([128, 2], f32, name="rec2", tag="rec2")
                        nc.vector.reciprocal(rec2[:], ps_o[:, 64::65])
                        ocat = ocp.tile([128, 128], bf16, name="ocat", tag="ocat")
                        nc.vector.tensor_scalar_mul(ocat[:, 0:64], ps_o[:, 0:64],
                                                    rec2[:, 0:1])
                        nc.vector.tensor_scalar_mul(ocat[:, 64:128], ps_o[:, 65:129],
                                                    rec2[:, 1:2])
                        return ocat

                    def _out(qt, ocat):
                        nc.sync.dma_start(
                            a2a_in[hh][b * 4 + qt // 4,
                                       (qt % 4) * 128:(qt % 4 + 1) * 128, :],
                            ocat[:],
                        )

                    _scores(0)
                    oc_prev = None
                    for kt in range(1, NT + 2):
                        if kt < NT:
                            _scores(kt)
                        if oc_prev is not None:
                            _out(kt - 2, oc_prev)
                            oc_prev = None
                        if kt - 1 < NT:
                            oc_prev = _av(kt - 1)

                # ---- A2A chunk for this head (overlaps the other head's work) ----
                nc.gpsimd.collective_compute(
                    "AllToAll",
                    mb.AluOpType.bypass,
                    replica_groups=[list(range(NCORES))],
                    ins=[a2a_in[hh][:].opt()],
                    outs=[a2a_out[hh][:].opt()],
                )

            # ---- mix: two accumulation waves, one per A2A chunk ----
            for hh in range(HLOC):
                gat_sb = constp.tile([128, 8, SB], bf16, name=f"gat_sb{hh}")
                for j in range(NCORES):
                    nc.scalar.dma_start_transpose(gat_sb[:, j, :], a2a_out[hh][j])
                for ec in range(2):
                    ps_ys = [psm.tile([128, 512], f32, name="ps_y", tag="psm")
                             for _ in range(4)]
                    for fo in range(8):
                        for st in range(4):
                            nc.tensor.matmul(
                                ps_ys[st][:],
                                gat_sb[:, fo, st * 128:(st + 1) * 128],
                                wmix_sb[:, hh * 8 + fo, ec * 512:(ec + 1) * 512],
                                start=(fo == 0),
                                stop=(fo == 7),
                            )
                    for st in range(4):
                        if hh == 0:
                            ysb = ysbp.tile([128, 512], f32, name="y0",
                                            tag=f"y0_{st}_{ec}")
                            nc.any.tensor_copy(ysb[:], ps_ys[st][:])
                            y0_sb.append(ysb)
                        else:
                            ysb = y0_sb[ec * 4 + st]
                            yout = tmpp.tile([128, 512], f32, name="yout", tag="yout")
                            nc.vector.tensor_add(yout[:], ysb[:], ps_ys[st][:])
                            nc.sync.dma_start(
                                y_d.ap()[st * 128:(st + 1) * 128,
                                         ec * 512:(ec + 1) * 512],
                                yout[:],
                            )

    nc.compile()
    return nc


# ----------------------------------------------------------------------------
# host wrapper
# ----------------------------------------------------------------------------

def _numpy_reference(x, w_qkv, w_mix, stride):
    """Fallback for unexpected shapes/stride: direct numpy port of the reference."""
    x = np.asarray(x, np.float32)
    Bx, Sx, Ex = x.shape
    Hx = 16
    Dx = Ex // Hx
    stride = int(stride)
    qkv = x @ np.asarray(w_qkv, np.float32).T
    qkv = qkv.reshape(Bx, Sx, 3, Hx, Dx).transpose(2, 0, 3, 1, 4)
    q, k, v = qkv[0], qkv[1], qkv[2]
    inv = 1.0 / (10000.0 ** (np.arange(0, Dx, 2, dtype=np.float32) / Dx))
    pos = np.arange(Sx, dtype=np.float32)
    emb = np.concatenate([pos[:, None] * inv[None, :]] * 2, axis=-1)
    cos, sin = np.cos(emb)[None, None], np.sin(emb)[None, None]

    def rot(t):
        t1 = t[..., ::2]
        t2 = t[..., 1::2]
        return np.stack((-t2, t1), axis=-1).reshape(t.shape)

    q = q * cos + rot(q) * sin
    k = k * cos + rot(k) * sin

    def compress(t):
        chunks = -(-Sx // stride)
        pad = chunks * stride - Sx
        if pad:
            t = np.concatenate([t, np.broadcast_to(t[:, :, -1:, :], t.shape[:2] + (pad, Dx))], axis=2)
        return t.reshape(Bx, Hx, chunks, stride, Dx).mean(axis=3)

    scale = 1.0 / np.sqrt(Dx)
    k_lat, v_lat = compress(k), compress(v)
    Cx = k_lat.shape[2]
    sc = np.einsum("bhsd,bhcd->bhsc", q, k_lat) * scale
    chunk_start = np.minimum(np.arange(Cx) * stride, Sx - 1)
    qpos = np.arange(Sx)
    mask = chunk_start[None, :] > qpos[:, None]
    sc = np.where(mask[None, None], -np.inf, sc)
    sc = sc - sc.max(axis=-1, keepdims=True)
    w = np.exp(sc)
    w = w / w.sum(axis=-1, keepdims=True)
    w = np.nan_to_num(w)
    compressed = np.einsum("bhsc,bhcd->bhsd", w, v_lat)

    kpos = np.arange(Sx)
    blocked = (kpos[None, :] > qpos[:, None]) | (kpos[None, :] < qpos[:, None] - stride + 1)
    ls = np.einsum("bhsd,bhtd->bhst", q, k) * scale
    ls = np.where(blocked[None, None], -np.inf, ls)
    ls = ls - ls.max(axis=-1, keepdims=True)
    lw = np.exp(ls)
    lw = lw / lw.sum(axis=-1, keepdims=True)
    local = np.einsum("bhst,bhtd->bhsd", lw, v)
    out = np.concatenate([compressed, local], axis=-1)
    out = out.transpose(0, 2, 1, 3).reshape(Bx, Sx, 2 * Ex)
    return (out @ np.asarray(w_mix, np.float32).T).astype(np.float32)


def _make_in_maps(x, w_qkv, w_mix):
    consts = _host_constants()
    # [chunk, p, eo, s]: chunk = b*4 + sc; e = eo*128 + p; s local to 512-chunk
    xT = np.ascontiguousarray(
        np.asarray(x, np.float32).reshape(2, 4, 512, 8, 128).transpose(0, 1, 4, 3, 2)
        .reshape(8, 128, 8, 512)
    ).astype(BF16)
    # w_mix.T rows permuted into wave order: even heads (hh=0), then odd heads
    wmixT = np.asarray(w_mix, np.float32).T  # [2048, 1024], row f = h*128 + r
    perm = []
    for hh in range(HLOC):
        for j in range(NCORES):
            h = HLOC * j + hh
            perm.extend(range(h * 128, (h + 1) * 128))
    wmixT = np.ascontiguousarray(
        wmixT[np.asarray(perm)].reshape(16, 128, 1024).transpose(1, 0, 2)
    ).astype(BF16)
    in_maps = []
    for c in range(NCORES):
        h0 = HLOC * c
        rows_q = slice(h0 * D, (h0 + HLOC) * D)
        rows_k = slice(E + h0 * D, E + (h0 + HLOC) * D)
        rows_v = slice(2 * E + h0 * D, 2 * E + (h0 + HLOC) * D)
        wqkT = np.ascontiguousarray(
            np.concatenate([w_qkv[rows_q], w_qkv[rows_k]], axis=0).T
            .reshape(8, 128, 256).transpose(1, 0, 2)
        ).astype(BF16)
        wvT = np.ascontiguousarray(
            w_qkv[rows_v].T.reshape(8, 128, 128).transpose(1, 0, 2)
        ).astype(BF16)
        in_maps.append({
            "xT": xT,
            "wqkT": wqkT,
            "wvT": wvT,
            "wmixT": wmixT,
            "cos2": consts["cos2"],
            "sin2": consts["sin2"],
            "r2": consts["r2"],
            "mloc": consts["mloc"],
            "latm": consts["latm"],
            "amat": consts["amat"],
            "identb": consts["identb"],
        })
    return in_maps


_LDW_PATCHED = []


def _enable_ldw_opt():
    """Flip walrus --enable-ldw-opt to true (elides redundant LDWEIGHTS)."""
    if _LDW_PATCHED:
        return
    import concourse.bass_utils as bu

    orig = bu.run_command

    def patched(cmd, *a, **kw):
        pass  # ldw-opt crashes walrus in this toolchain; keep default
        return orig(cmd, *a, **kw)

    bu.run_command = patched
    _LDW_PATCHED.append(True)


def run_device(x, w_qkv, w_mix, trace=False, **spmd_kwargs):
    from concourse.bass_utils import run_bass_kernel_spmd
    _enable_ldw_opt()

    if "nc" not in _BUILD_CACHE:
        _BUILD_CACHE["nc"] = build_bass()
    nc = _BUILD_CACHE["nc"]
    in_maps = _make_in_maps(np.asarray(x, np.float32), np.asarray(w_qkv, np.float32),
                            np.asarray(w_mix, np.float32))
    res = run_bass_kernel_spmd(nc, in_maps, core_ids=list(range(NCORES)),
                               trace=trace, **spmd_kwargs)
    out = np.zeros((B, S, E), np.float32)
    for d in range(NCORES):
        bb, sq = d // 4, d % 4
        out[bb, sq * SB:(sq + 1) * SB] = np.asarray(res.results[d]["y"], np.float32)
    return out, res


def kernel(x, w_qkv, w_mix, stride):
    x = np.asarray(x)
    if int(stride) != STRIDE or x.shape != (B, S, E):
        return _numpy_reference(x, w_qkv, w_mix, stride)
    out, _ = run_device(x, w_qkv, w_mix)
    return out


# revision 25
# speedup vs baseline: 1.0102x; 1.0102x over previous
"""Trainium2 8-core Bass kernel for AdaptiveAttentionTransformerBlock.

Sparse attention with a latent (stride-64 mean-pooled KV) branch and a local
sliding-window (width 64) branch, concatenated per head and mixed by w_mix.

Sharding: 16 heads -> 2 heads per core (all 8 cores), both batches per core.
Each core computes qkv + RoPE + both attention branches for its 2 heads in a
transposed [feature, seq] layout; two 8-rank AllToAll chunks (one per local
head) redistribute from head-sharding to (batch, seq/4) sharding, overlapped
with the second head's attention; each core then computes the full mix
projection for its 512-token slice in two accumulation waves. The host
concatenates the 8 slices.

Self-contained: hardcodes shapes B=2, S=2048, E=1024, H=16, D=64, stride=64.
"""

import os
import sys

import numpy as np

for _p in ("/opt/trn_rl_repo",):
    if _p not in sys.path and os.path.isdir(_p):
        sys.path.insert(0, _p)

import ml_dtypes  # noqa: E402

B, S, E, H, D = 2, 2048, 1024, 16, 64
STRIDE, C = 64, 32
NCORES, HLOC = 8, 2
SB = S // 4  # 512, per-core output seq slice
NT = S // 128  # 16 q/k tiles
BF16 = ml_dtypes.bfloat16

_BUILD_CACHE = {}


# ----------------------------------------------------------------------------
# host-side constants
# ----------------------------------------------------------------------------

def _host_constants():
    inv = 1.0 / (10000.0 ** (np.arange(0, D, 2, dtype=np.float64) / D))
    pos = np.arange(S, dtype=np.float64)
    ang = pos[None, :] * inv[(np.arange(D) % (D // 2))][:, None]  # [64, S]
    cosT = np.cos(ang).astype(np.float32)
    sinT = np.sin(ang).astype(np.float32)
    cos2 = np.vstack([cosT, cosT])  # [128, S] f32
    sin2 = np.vstack([sinT, sinT])

    L = np.zeros((D, D), np.float32)
    for j in range(D // 2):
        L[2 * j + 1, 2 * j] = -1.0
        L[2 * j, 2 * j + 1] = 1.0
    L2 = np.zeros((2 * D, 2 * D), np.float32)
    L2[:D, :D] = L
    L2[D:, D:] = L

    # combined local mask for scores_T of key-tile kt vs q-tiles [kt, kt+1]:
    # cols 0:128 (q in same tile): key j visible iff j <= i <= j+63
    # cols 128:256 (q in next tile): key j visible iff j >= i + 65
    i = np.arange(128)
    mdiag = ((i[None, :] >= i[:, None]) & (i[None, :] - i[:, None] <= 63)).astype(np.float32)
    mprev = (i[:, None] >= i[None, :] + 65).astype(np.float32)
    mloc = np.concatenate([mdiag, mprev], axis=1)  # [128, 256]

    cc = np.arange(C)
    ss = np.arange(S)
    latm = (cc[:, None] * STRIDE <= ss[None, :]).astype(np.float32)  # [32, S]

    A = np.zeros((S, C), np.float32)
    for c in range(C):
        A[c * STRIDE:(c + 1) * STRIDE, c] = 1.0 / STRIDE

    ident = np.eye(128, dtype=np.float32)
    return {
        "cos2": cos2,
        "sin2": sin2,
        "r2": L2.astype(BF16),
        "mloc": mloc.astype(BF16),
        "latm": latm.astype(BF16),
        "amat": np.ascontiguousarray(
            A.reshape(16, 128, C).transpose(1, 0, 2)).astype(BF16),
        "identf": ident,
        "identb": ident.astype(BF16),
    }


# ----------------------------------------------------------------------------
# bass graph
# ----------------------------------------------------------------------------

def build_bass():
    import concourse.mybir as mb
    import concourse.tile as tile
    from concourse import bacc

    f32 = mb.dt.float32
    bf16 = mb.dt.bfloat16

    nc = bacc.Bacc(
        "TRN2",
        target_bir_lowering=False,
        debug=False,
        num_devices=NCORES,
    )

    xT_d = nc.declare_dram_parameter("xT", [8, 128, 8, 512], bf16, isOutput=False)
    wqkT_d = nc.declare_dram_parameter("wqkT", [128, 8, 256], bf16, isOutput=False)
    wvT_d = nc.declare_dram_parameter("wvT", [128, 8, 128], bf16, isOutput=False)
    # w_mix.T with rows permuted into the two A2A wave orders (even heads, odd heads)
    wmixT_d = nc.declare_dram_parameter("wmixT", [128, 16, E], bf16, isOutput=False)
    cos2_d = nc.declare_dram_parameter("cos2", [128, S], f32, isOutput=False)
    identb_d = nc.declare_dram_parameter("identb", [128, 128], bf16, isOutput=False)
    sin2_d = nc.declare_dram_parameter("sin2", [128, S], f32, isOutput=False)
    r2_d = nc.declare_dram_parameter("r2", [128, 128], bf16, isOutput=False)
    mloc_d = nc.declare_dram_parameter("mloc", [128, 256], bf16, isOutput=False)
    latm_d = nc.declare_dram_parameter("latm", [C, S], bf16, isOutput=False)
    amat_d = nc.declare_dram_parameter("amat", [128, 16, C], bf16, isOutput=False)
    y_d = nc.declare_dram_parameter("y", [SB, E], f32, isOutput=True)

    SCALE = 1.0 / 8.0  # 1/sqrt(D)

    with tile.TileContext(nc, num_cores=NCORES) as tc:
        with (
            tc.tile_pool(name="const", bufs=1) as constp,
            tc.tile_pool(name="xt", bufs=1) as xtp,
            tc.tile_pool(name="qk", bufs=1) as qkp,
            tc.tile_pool(name="vex", bufs=1) as vexp,
            tc.tile_pool(name="lat", bufs=1) as latp,
            tc.tile_pool(name="plat", bufs=3) as platp,
            tc.tile_pool(name="pk", bufs=3) as pkp,
            tc.tile_pool(name="tmp", bufs=3) as tmpp,
            tc.tile_pool(name="oc", bufs=4) as ocp,
            tc.tile_pool(name="ysb", bufs=1) as ysbp,
            tc.tile_pool(name="psq", bufs=2, space="PSUM") as psq,
            tc.tile_pool(name="psm", bufs=6, space="PSUM") as psm,
            tc.tile_pool(name="dram", bufs=1, space="DRAM") as dramp,
        ):
            # ---- critical-path loads first: qk weights + x chunks ----
            wqk_sb = constp.tile([128, 8, 256], bf16, name="wqk_sb")
            nc.sync.dma_start(wqk_sb[:], wqkT_d.ap())
            xt_tiles = []
            for i in range(8):
                t = xtp.tile([128, 8, 512], bf16, name=f"xt{i}", tag="xt", bufs=6)
                nc.sync.dma_start(t[:], xT_d.ap()[i])
                xt_tiles.append(t)
            wv_sb = constp.tile([128, 8, 128], bf16, name="wv_sb")
            nc.sync.dma_start(wv_sb[:], wvT_d.ap())
            r2_sb = constp.tile([128, 128], bf16, name="r2_sb")
            nc.gpsimd.dma_start(r2_sb[:], r2_d.ap())
            cos_sb = constp.tile([128, S], f32, name="cos_sb")
            nc.gpsimd.dma_start(cos_sb[:], cos2_d.ap())
            sin_sb = constp.tile([128, S], f32, name="sin_sb")
            nc.gpsimd.dma_start(sin_sb[:], sin2_d.ap())
            identb_sb = constp.tile([128, 128], bf16, name="identb_sb")
            nc.gpsimd.dma_start(identb_sb[:], identb_d.ap())
            amat_sb = constp.tile([128, 16, C], bf16, name="amat_sb")
            nc.gpsimd.dma_start(amat_sb[:], amat_d.ap())
            mloc_sb = constp.tile([128, 256], bf16, name="mloc_sb")
            nc.gpsimd.dma_start(mloc_sb[:], mloc_d.ap())
            latm_sb = constp.tile([C, S], bf16, name="latm_sb")
            nc.gpsimd.dma_start(latm_sb[:], latm_d.ap())
            wmix_sb = constp.tile([128, 16, E], bf16, name="wmix_sb")
            nc.sync.dma_start(wmix_sb[:], wmixT_d.ap())

            a2a_in = [dramp.tile([NCORES, SB, 128], bf16, name=f"a2a_in{h}") for h in range(HLOC)]
            a2a_out = [dramp.tile([NCORES, SB, 128], bf16, name=f"a2a_out{h}") for h in range(HLOC)]

            q_rot, k_rot, v_ext, klat2 = [], [], [], []
            vlat = [[None] * HLOC for _ in range(B)]

            # ================= qkv + rope (both batches) =================
            for b in range(B):
                q_rot.append(qkp.tile([128, S], bf16, name=f"q_rot{b}", tag=f"q_rot{b}"))
                k_rot.append(qkp.tile([128, S], bf16, name=f"k_rot{b}", tag=f"k_rot{b}"))
                pending_rope = []

                def _rope_tail(dst, sc, ps_qk):
                    tmp_bf = tmpp.tile([128, 512], bf16, name="tmp_bf", tag="tmp_bf")
                    nc.any.tensor_copy(tmp_bf[:], ps_qk[:])
                    ps_rh = psm.tile([128, 512], f32, name="ps_rh", tag="psm")
                    nc.tensor.matmul(ps_rh[:], r2_sb[:], tmp_bf[:], start=True, stop=True)
                    ssl = slice(sc * 512, (sc + 1) * 512)
                    t1 = tmpp.tile([128, 512], f32, name="t1", tag="t1")
                    nc.vector.tensor_mul(t1[:], ps_qk[:], cos_sb[:, ssl])
                    t2 = tmpp.tile([128, 512], f32, name="t2", tag="t2")
                    nc.vector.tensor_mul(t2[:], ps_rh[:], sin_sb[:, ssl])
                    nc.vector.tensor_add(dst[:, ssl], t1[:], t2[:])

                for jt in range(2):  # 0: q, 1: k
                    dst = (q_rot if jt == 0 else k_rot)[b]
                    for sc in range(4):
                        xt = xt_tiles[b * 4 + sc]
                        ps_qk = psq.tile([128, 512], f32, name="ps_qk", tag="psq")
                        for e in range(8):
                            nc.tensor.matmul(
                                ps_qk[:],
                                wqk_sb[:, e, jt * 128:(jt + 1) * 128],
                                xt[:, e, :],
                                start=(e == 0),
                                stop=(e == 7),
                            )
                        pending_rope.append((dst, sc, ps_qk))
                        if len(pending_rope) > 1:
                            _rope_tail(*pending_rope.pop(0))
                while pending_rope:
                    _rope_tail(*pending_rope.pop(0))

                # v^T then PE-transpose into v_ext [s-part, 16, 2 heads, 64+1]
                ve = vexp.tile([128, 16, HLOC, 65], bf16, name=f"v_ext{b}", tag=f"v_ext{b}")
                v_ext.append(ve)
                nc.any.memset(ve[:, :, :, 64], 1.0)
                pending_vt = []

                def _v_tail(sc, vT_bf):
                    for sub in range(4):
                        st = sc * 4 + sub
                        ps_tv = psm.tile([128, 128], bf16, name="ps_tv", tag="psm")
                        nc.tensor.transpose(
                            ps_tv[:], vT_bf[:, sub * 128:(sub + 1) * 128], identb_sb[:]
                        )
                        nc.any.tensor_copy(
                            ve[:, st, :, 0:64],
                            ps_tv[:].rearrange("p (h d) -> p h d", h=2),
                        )

                for sc in range(4):
                    xt = xt_tiles[b * 4 + sc]
                    ps_vT = psq.tile([128, 512], f32, name="ps_vT", tag="psq")
                    for e in range(8):
                        nc.tensor.matmul(
                            ps_vT[:],
                            wv_sb[:, e, :],
                            xt[:, e, :],
                            start=(e == 0),
                            stop=(e == 7),
                        )
                    vT_bf = tmpp.tile([128, 512], bf16, name="vT_bf", tag="vT_bf")
                    nc.any.tensor_copy(vT_bf[:], ps_vT[:])
                    pending_vt.append((sc, vT_bf))
                    if len(pending_vt) > 1:
                        _v_tail(*pending_vt.pop(0))
                while pending_vt:
                    _v_tail(*pending_vt.pop(0))

                # latent k: chunk means for both heads at their partition offsets
                kl2 = latp.tile([128, C], bf16, name=f"klat2{b}", tag=f"klat2{b}")
                klat2.append(kl2)
                kl32 = latp.tile([128, C], f32, name=f"kl32{b}", tag="kl32")
                nc.vector.tensor_reduce(
                    out=kl32[:],
                    in_=k_rot[b][:].rearrange("p (c w) -> p c w", w=STRIDE),
                    op=mb.AluOpType.add,
                    axis=mb.AxisListType.X,
                )
                nc.scalar.activation(kl2[:], kl32[:], mb.ActivationFunctionType.Copy,
                                     bias=0.0, scale=1.0 / STRIDE)

                # latent v (+ones col) per head: vlat_ext[c, 65] via A-matmul
                for hh in range(HLOC):
                    ps_vl = psm.tile([C, 65], f32, name="ps_vl", tag="psm")
                    for st in range(16):
                        nc.tensor.matmul(
                            ps_vl[:],
                            amat_sb[:, st, :],
                            v_ext[b][:, st, hh, :],
                            start=(st == 0),
                            stop=(st == 15),
                        )
                    vl = latp.tile([C, 65], bf16, name=f"vlat{b}{hh}", tag=f"vlat{b}{hh}")
                    nc.any.tensor_copy(vl[:], ps_vl[:])
                    vlat[b][hh] = vl

            # ================= attention (hh-major) + chunked A2A + mix ====
            y0_sb = []  # wave-0 partial mix results
            for hh in range(HLOC):
                hsl = slice(hh * D, (hh + 1) * D)
                for b in range(B):
                    # latent probabilities for all queries of this (b, head)
                    p_lat = platp.tile([C, S], bf16, name="p_lat", tag="p_lat")
                    for qc in range(4):
                        qsl512 = slice(qc * 512, (qc + 1) * 512)
                        ps_ls = psm.tile([C, 512], f32, name="ps_ls", tag="psm")
                        nc.tensor.matmul(ps_ls[:], klat2[b][hsl, :], q_rot[b][hsl, qsl512],
                                         start=True, stop=True)
                        nc.scalar.activation(p_lat[:, qsl512], ps_ls[:],
                                             mb.ActivationFunctionType.Exp,
                                             bias=0.0, scale=SCALE)
                        nc.vector.tensor_mul(p_lat[:, qsl512], p_lat[:, qsl512],
                                             latm_sb[:, qsl512])

                    # software-pipelined: scores one tile ahead, output one behind
                    pk = {}

                    def _scores(kt):
                        qn = 256 if kt < NT - 1 else 128
                        ps_s = psm.tile([128, 256], f32, name="ps_s", tag="psm")
                        nc.tensor.matmul(
                            ps_s[:, :qn],
                            k_rot[b][hsl, kt * 128:(kt + 1) * 128],
                            q_rot[b][hsl, kt * 128:kt * 128 + qn],
                            start=True, stop=True,
                        )
                        p_kt = pkp.tile([128, 256], bf16, name="p_kt", tag="p_kt")
                        nc.scalar.activation(p_kt[:, :qn], ps_s[:, :qn],
                                             mb.ActivationFunctionType.Exp,
                                             bias=0.0, scale=SCALE)
                        nc.vector.tensor_mul(p_kt[:, :qn], p_kt[:, :qn], mloc_sb[:, :qn])
                        pk[kt] = p_kt

                    def _av(qt):
                        qsl = slice(qt * 128, (qt + 1) * 128)
                        ps_o = psm.tile([128, 130], f32, name="ps_o", tag="psm")
                        nc.tensor.matmul(ps_o[:, 0:65], p_lat[:, qsl], vlat[b][hh][:],
                                         start=True, stop=True)
                        if qt > 0:
                            nc.tensor.matmul(ps_o[:, 65:130], pk[qt - 1][:, 128:256],
                                             v_ext[b][:, qt - 1, hh, :],
                                             start=True, stop=False)
                        nc.tensor.matmul(ps_o[:, 65:130], pk[qt][:, 0:128],
                                         v_ext[b][:, qt, hh, :],
                                         start=(qt == 0), stop=True)
                        if qt > 0:
                            del pk[qt - 1]
                        # normalize -> ocat [128 q, 128 (cmp|loc)] bf16 (DVE only)
                        rec2 = tmpp.tile([128, 2], f32, name="rec2", tag="rec2")
                        nc.vector.reciprocal(rec2[:], ps_o[:, 64::65])
                        ocat = ocp.tile([128, 128], bf16, name="ocat", tag="ocat")
                        nc.vector.tensor_scalar_mul(ocat[:, 0:64], ps_o[:, 0:64],
                                                    rec2[:, 0:1])
                        nc.vector.tensor_scalar_mul(ocat[:, 64:128], ps_o[:, 65:129],
                                                    rec2[:, 1:2])
                        return ocat

                    def _out(qt, ocat):
                        nc.sync.dma_start(
                            a2a_in[hh][b * 4 + qt // 4,
                                       (qt % 4) * 128:(qt % 4 + 1) * 128, :],
                            ocat[:],
                        )

                    _scores(0)
                    oc_prev = None
                    for kt in range(1, NT + 2):
                        if kt < NT:
                            _scores(kt)
                        if oc_prev is not None:
                            _out(kt - 2, oc_prev)
                            oc_prev = None
                        if kt - 1 < NT:
                            oc_prev = _av(kt - 1)

                # ---- A2A chunk for this head (overlaps the other head's work) ----
                nc.gpsimd.collective_compute(
                    "AllToAll",
                    mb.AluOpType.bypass,
                    replica_groups=[list(range(NCORES))],
                    ins=[a2a_in[hh][:].opt()],
                    outs=[a2a_out[hh][:].opt()],
                )

            # ---- mix: two accumulation waves, one per A2A chunk ----
            for hh in range(HLOC):
                gat_sb = constp.tile([128, 8, SB], bf16, name=f"gat_sb{hh}")
                for j in range(NCORES):
                    nc.scalar.dma_start_transpose(gat_sb[:, j, :], a2a_out[hh][j])
                for ec in range(2):
                    ps_ys = [psm.tile([128, 512], f32, name="ps_y", tag="psm")
                             for _ in range(4)]
                    for fo in range(8):
                        for st in range(4):
                            nc.tensor.matmul(
                                ps_ys[st][:],
                                gat_sb[:, fo, st * 128:(st + 1) * 128],
                                wmix_sb[:, hh * 8 + fo, ec * 512:(ec + 1) * 512],
                                start=(fo == 0),
                                stop=(fo == 7),
                            )
                    for st in range(4):
                        if hh == 0:
                            ysb = ysbp.tile([128, 512], f32, name="y0",
                                            tag=f"y0_{st}_{ec}")
                            nc.any.tensor_copy(ysb[:], ps_ys[st][:])
                            y0_sb.append(ysb)
                        else:
                            ysb = y0_sb[ec * 4 + st]
                            yout = tmpp.tile([128, 512], f32, name="yout", tag="yout")
                            nc.vector.tensor_add(yout[:], ysb[:], ps_ys[st][:])
                            nc.sync.dma_start(
                                y_d.ap()[st * 128:(st + 1) * 128,
                                         ec * 512:(ec + 1) * 512],
                                yout[:],
                            )

    nc.compile()
    return nc


# ----------------------------------------------------------------------------
# host wrapper
# ----------------------------------------------------------------------------

def _numpy_reference(x, w_qkv, w_mix, stride):
    """Fallback for unexpected shapes/stride: direct numpy port of the reference."""
    x = np.asarray(x, np.float32)
    Bx, Sx, Ex = x.shape
    Hx = 16
    Dx = Ex // Hx
    stride = int(stride)
    qkv = x @ np.asarray(w_qkv, np.float32).T
    qkv = qkv.reshape(Bx, Sx, 3, Hx, Dx).transpose(2, 0, 3, 1, 4)
    q, k, v = qkv[0], qkv[1], qkv[2]
    inv = 1.0 / (10000.0 ** (np.arange(0, Dx, 2, dtype=np.float32) / Dx))
    pos = np.arange(Sx, dtype=np.float32)
    emb = np.concatenate([pos[:, None] * inv[None, :]] * 2, axis=-1)
    cos, sin = np.cos(emb)[None, None], np.sin(emb)[None, None]

    def rot(t):
        t1 = t[..., ::2]
        t2 = t[..., 1::2]
        return np.stack((-t2, t1), axis=-1).reshape(t.shape)

    q = q * cos + rot(q) * sin
    k = k * cos + rot(k) * sin

    def compress(t):
        chunks = -(-Sx // stride)
        pad = chunks * stride - Sx
        if pad:
            t = np.concatenate([t, np.broadcast_to(t[:, :, -1:, :], t.shape[:2] + (pad, Dx))], axis=2)
        return t.reshape(Bx, Hx, chunks, stride, Dx).mean(axis=3)

    scale = 1.0 / np.sqrt(Dx)
    k_lat, v_lat = compress(k), compress(v)
    Cx = k_lat.shape[2]
    sc = np.einsum("bhsd,bhcd->bhsc", q, k_lat) * scale
    chunk_start = np.minimum(np.arange(Cx) * stride, Sx - 1)
    qpos = np.arange(Sx)
    mask = chunk_start[None, :] > qpos[:, None]
    sc = np.where(mask[None, None], -np.inf, sc)
    sc = sc - sc.max(axis=-1, keepdims=True)
    w = np.exp(sc)
    w = w / w.sum(axis=-1, keepdims=True)
    w = np.nan_to_num(w)
    compressed = np.einsum("bhsc,bhcd->bhsd", w, v_lat)

    kpos = np.arange(Sx)
    blocked = (kpos[None, :] > qpos[:, None]) | (kpos[None, :] < qpos[:, None] - stride + 1)
    ls = np.einsum("bhsd,bhtd->bhst", q, k) * scale
    ls = np.where(blocked[None, None], -np.inf, ls)
    ls = ls - ls.max(axis=-1, keepdims=True)
    lw = np.exp(ls)
    lw = lw / lw.sum(axis=-1, keepdims=True)
    local = np.einsum("bhst,bhtd->bhsd", lw, v)
    out = np.concatenate([compressed, local], axis=-1)
    out = out.transpose(0, 2, 1, 3).reshape(Bx, Sx, 2 * Ex)
    return (out @ np.asarray(w_mix, np.float32).T).astype(np.float32)


def _make_in_maps(x, w_qkv, w_mix):
    consts = _host_constants()
    # [chunk, p, eo, s]: chunk = b*4 + sc; e = eo*128 + p; s local to 512-chunk
    xT = np.ascontiguousarray(
        np.asarray(x, np.float32).reshape(2, 4, 512, 8, 128).transpose(0, 1, 4, 3, 2)
        .reshape(8, 128, 8, 512)
    ).astype(BF16)
    # w_mix.T rows permuted into wave order: even heads (hh=0), then odd heads
    wmixT = np.asarray(w_mix, np.float32).T  # [2048, 1024], row f = h*128 + r
    perm = []
    for hh in range(HLOC):
        for j in range(NCORES):
            h = HLOC * j + hh
            perm.extend(range(h * 128, (h + 1) * 128))
    wmixT = np.ascontiguousarray(
        wmixT[np.asarray(perm)].reshape(16, 128, 1024).transpose(1, 0, 2)
    ).astype(BF16)
    in_maps = []
    for c in range(NCORES):
        h0 = HLOC * c
        rows_q = slice(h0 * D, (h0 + HLOC) * D)
        rows_k = slice(E + h0 * D, E + (h0 + HLOC) * D)
        rows_v = slice(2 * E + h0 * D, 2 * E + (h0 + HLOC) * D)
        wqkT = np.ascontiguousarray(
            np.concatenate([w_qkv[rows_q], w_qkv[rows_k]], axis=0).T
            .reshape(8, 128, 256).transpose(1, 0, 2)
        ).astype(BF16)
        wvT = np.ascontiguousarray(
            w_qkv[rows_v].T.reshape(8, 128, 128).transpose(1, 0, 2)
        ).astype(BF16)
        in_maps.append({
            "xT": xT,
            "wqkT": wqkT,
            "wvT": wvT,
            "wmixT": wmixT,
            "cos2": consts["cos2"],
            "sin2": consts["sin2"],
            "r2": consts["r2"],
            "mloc": consts["mloc"],
            "latm": consts["latm"],
            "amat": consts["amat"],
            "identb": consts["identb"],
        })
    return in_maps


_LDW_PATCHED = []


def _enable_ldw_opt():
    """Flip walrus --enable-ldw-opt to true (elides redundant LDWEIGHTS)."""
    if _LDW_PATCHED:
        return
    import concourse.bass_utils as bu

    orig = bu.run_command

    def patched(cmd, *a, **kw):
        pass  # ldw-opt crashes walrus in this toolchain; keep default
        return orig(cmd, *a, **kw)

    bu.run_command = patched
    _LDW_PATCHED.append(True)


def run_device(x, w_qkv, w_mix, trace=False, **spmd_kwargs):
    from concourse.bass_utils import run_bass_kernel_spmd
    _enable_ldw_opt()

    if "nc" not in _BUILD_CACHE:
        _BUILD_CACHE["nc"] = build_bass()
    nc = _BUILD_CACHE["nc"]
    in_maps = _make_in_maps(np.asarray(x, np.float32), np.asarray(w_qkv, np.float32),
                            np.asarray(w_mix, np.float32))
    res = run_bass_kernel_spmd(nc, in_maps, core_ids=list(range(NCORES)),
                               trace=trace, **spmd_kwargs)
    out = np.zeros((B, S, E), np.float32)
    for d in range(NCORES):
        bb, sq = d // 4, d % 4
        out[bb, sq * SB:(sq + 1) * SB] = np.asarray(res.results[d]["y"], np.float32)
    return out, res


def kernel(x, w_qkv, w_mix, stride):
    x = np.asarray(x)
    if int(stride) != STRIDE or x.shape != (B, S, E):
        return _numpy_reference(x, w_qkv, w_mix, stride)
    out, _ = run_device(x, w_qkv, w_mix)
    return out
